# revision 3
# baseline (speedup 1.0000x reference)
"""Stereo cost-volume construction kernel for Trainium2 (8 NeuronCores).

Problem: left, right: [B=4, C=32, H=64, W=128] f32 ->
         cost:        [B, 2C=64, D=48, H, W] f32
  cost[b, c,    d, h, w] = left [b, c, h, w]     if w >= d else 0
  cost[b, C+c,  d, h, w] = right[b, c, h, w - d] if w >= d else 0

Sharding: data-parallel over (b, h-half): core = b*2 + hh, each core owns
the full disparity range on a [C, 32, W] slice -> pure SPMD, no
communication, identical program on all 8 cores.

Per-core device strategy (pure DMA, memory-regime):
  * right half: SBUF holds an "extended" image ext[p, j, 0:2W] per row
    (= 128 zeros ++ the row of `right`). The full output row for
    disparity d is exactly the contiguous 512-B window ext[.., W-d : 2W-d]
    -> one 512 KiB DMA per d, zero compute, full-width writes.
  * left half: K rotating SBUF stage buffers holding `left` with a
    growing zero column-prefix. Stage k serves d = k, k+K, k+2K, ...;
    between uses only the K newly-masked columns are memset (DVE).
    -> one 512 KiB full-width DMA per d + tiny memsets.
All output bytes are written exactly once with >=512B contiguous runs.
"""

import numpy as np

import concourse.bass as bass
import concourse.mybir as mybir
from concourse.bass_utils import run_bass_kernel_spmd

B, C, H, W = 4, 32, 64, 128
D = 48
HH = H // 2          # rows of H per core
N_CORES = 8
ROWS = C * HH        # 1024 (c, h) rows per core
P = 128              # SBUF partitions
J = ROWS // P        # 8 rows per partition
K = 4                # left stage buffers
F32 = mybir.dt.float32


def _build_nc() -> bass.Bass:
    nc = bass.Bass()

    left_t = nc.declare_dram_parameter("left", [ROWS, W], F32, isOutput=False)
    right_t = nc.declare_dram_parameter("right", [ROWS, W], F32, isOutput=False)
    out_t = nc.declare_dram_parameter("out", [2 * C, D, HH, W], F32, isOutput=True)

    ext = nc.alloc_sbuf_tensor("ext", [P, J, 2 * W], F32)
    stages = [nc.alloc_sbuf_tensor(f"stage{k}", [P, J, W], F32) for k in range(K)]

    s_ld = nc.alloc_semaphore("s_ld")      # left stage loads
    s_ext = nc.alloc_semaphore("s_ext")    # ext data load
    s_zext = nc.alloc_semaphore("s_zext")  # ext zero-half memset
    s_ms = nc.alloc_semaphore("s_ms")      # left stage memsets, one per mask level
    s_r = nc.alloc_semaphore("s_r")        # right output DMAs
    s_lk = [nc.alloc_semaphore(f"s_l{k}") for k in range(K)]  # left out DMAs per stage

    uses = [len(range(k, D, K)) for k in range(K)]

    with nc.Block() as block:

        @block.vector
        def _(v):
            # static zero half of ext: ext[.., 0:W] = 0 (once)
            v.memset(ext[:, :, 0:W], 0.0).then_inc(s_zext, 1)
            # initial masks for stages 1..K-1 (stage k starts at level k).
            # Wait for ALL K loads: same-queue DMA completions are unordered,
            # so a per-load sem count can't identify which stage is loaded.
            v.wait_ge(s_ld, 16 * K)
            for k in range(1, K):
                v.memset(stages[k][:, :, 0:k], 0.0).then_inc(s_ms, 1)
            # rolling masks: stage d%K goes from level d-K to level d
            for d in range(K, D):
                k = d % K
                v.wait_ge(s_lk[k], 16 * (d // K))  # last reader of this stage done
                v.memset(stages[k][:, :, d - K:d], 0.0).then_inc(s_ms, 1)

        @block.sync
        def _(s):
            for k in range(K):
                s.dma_start(out=stages[k][:], in_=left_t[:]).then_inc(s_ld, 16)
            for d in range(D):
                k = d % K
                if d == 0:
                    s.wait_ge(s_ld, 16 * K)
                if d >= 1:
                    s.wait_ge(s_ms, d)  # mask level d ready
                s.dma_start(
                    out=out_t[0:C, d:d + 1, :, :], in_=stages[k][:]
                ).then_inc(s_lk[k], 16)
            for k in range(K):
                s.wait_ge(s_lk[k], 16 * uses[k])

        @block.scalar
        def _(a):
            a.dma_start(out=ext[:, :, W:2 * W], in_=right_t[:]).then_inc(s_ext, 16)
            a.wait_ge(s_ext, 16)
            a.wait_ge(s_zext, 1)
            for d in range(D):
                a.dma_start(
                    out=out_t[C:2 * C, d:d + 1, :, :],
                    in_=ext[:, :, W - d:2 * W - d],
                ).then_inc(s_r, 16)
            a.wait_ge(s_r, 16 * D)

    return nc


_NC_CACHE: list = []


def _get_nc() -> bass.Bass:
    if not _NC_CACHE:
        _NC_CACHE.append(_build_nc())
    return _NC_CACHE[0]


def _shard(left: np.ndarray, right: np.ndarray) -> list:
    in_maps = []
    for b in range(B):
        for hh in range(H // HH):
            lc = np.ascontiguousarray(
                left[b, :, hh * HH:(hh + 1) * HH, :], dtype=np.float32
            ).reshape(ROWS, W)
            rc = np.ascontiguousarray(
                right[b, :, hh * HH:(hh + 1) * HH, :], dtype=np.float32
            ).reshape(ROWS, W)
            in_maps.append({"left": lc, "right": rc})
    return in_maps


def _run(left: np.ndarray, right: np.ndarray, **spmd_kwargs):
    nc = _get_nc()
    in_maps = _shard(left, right)
    res = run_bass_kernel_spmd(nc, in_maps, list(range(N_CORES)), **spmd_kwargs)
    out = np.empty((B, 2 * C, D, H, W), dtype=np.float32)
    core = 0
    for b in range(B):
        for hh in range(H // HH):
            out[b, :, :, hh * HH:(hh + 1) * HH, :] = res.results[core]["out"].reshape(
                2 * C, D, HH, W
            )
            core += 1
    return out, res


def kernel(left: np.ndarray, right: np.ndarray) -> np.ndarray:
    out, _ = _run(np.asarray(left), np.asarray(right))
    return out


# revision 5
# speedup vs baseline: 1.2032x; 1.2032x over previous
"""Stereo cost-volume construction kernel for Trainium2 (8 NeuronCores).

Problem: left, right: [B=4, C=32, H=64, W=128] f32 ->
         cost:        [B, 2C=64, D=48, H, W] f32
  cost[b, c,    d, h, w] = left [b, c, h, w]     if w >= d else 0
  cost[b, C+c,  d, h, w] = right[b, c, h, w - d] if w >= d else 0

Sharding: data-parallel over (b, h-half): core = b*2 + hh, each core owns
the full disparity range on a [C, 32, W] slice -> pure SPMD, no
communication, identical program on all 8 cores.

Per-core device strategy (memory-regime; all output bytes written
exactly once, full-width 512 KiB DMAs with 4 KiB descriptor runs):
  * K rotating SBUF stage buffers per half. A stage holds the complete
    output image for one disparity (zero prefix + data), so the output
    DMA is a plain full-width copy at peak descriptor efficiency.
  * left half: stage data never moves between uses (only the zero
    column-prefix grows), so reuse costs just a K-column memset
    (gpsimd); DVE seeds the K stages once from SBUF.
  * right half: data shifts with d, so DVE rebuilds each stage
    (memset of the K new prefix columns + shifted row copy).
  * left DMAs on the SP HWDGE queue, right DMAs on the Activation
    HWDGE queue; the two streams share the ~400 GB/s DMA engine pool.
"""

import numpy as np

import concourse.bass as bass
import concourse.mybir as mybir
from concourse.bass_utils import run_bass_kernel_spmd

B, C, H, W = 4, 32, 64, 128
D = 48
HH = H // 2          # rows of H per core
N_CORES = 8
ROWS = C * HH        # 1024 (c, h) rows per core
P = 128              # SBUF partitions
J = ROWS // P        # 8 rows per partition
K = 6                # stage buffers per half
F32 = mybir.dt.float32


def _build_nc() -> bass.Bass:
    nc = bass.Bass()

    left_t = nc.declare_dram_parameter("left", [ROWS, W], F32, isOutput=False)
    right_t = nc.declare_dram_parameter("right", [ROWS, W], F32, isOutput=False)
    out_t = nc.declare_dram_parameter("out", [2 * C, D, HH, W], F32, isOutput=True)

    lsb = nc.alloc_sbuf_tensor("lsb", [P, J, W], F32)
    rsb = nc.alloc_sbuf_tensor("rsb", [P, J, W], F32)
    lst = [nc.alloc_sbuf_tensor(f"lst{k}", [P, J, W], F32) for k in range(K)]
    rst = [nc.alloc_sbuf_tensor(f"rst{k}", [P, J, W], F32) for k in range(K)]

    s_lin = nc.alloc_semaphore("s_lin")
    s_rin = nc.alloc_semaphore("s_rin")
    s_prl_init = nc.alloc_semaphore("s_prl_init")  # DVE left seeds, d < K
    s_prl_roll = nc.alloc_semaphore("s_prl_roll")  # gpsimd left memsets, d >= K
    s_prr = nc.alloc_semaphore("s_prr")            # DVE right preps
    s_ldone = [nc.alloc_semaphore(f"s_ldone{k}") for k in range(K)]
    s_rdone = [nc.alloc_semaphore(f"s_rdone{k}") for k in range(K)]

    uses = [len(range(k, D, K)) for k in range(K)]

    with nc.Block() as block:

        @block.vector
        def _(v):
            # Seed left stages (data is d-invariant) and build right stages.
            # Interleave so both DMA queues start streaming ASAP.
            v.wait_ge(s_lin, 16)
            v.wait_ge(s_rin, 16)
            for k in range(K):
                if k > 0:
                    v.memset(lst[k][:, :, 0:k], 0.0)
                v.tensor_copy(out=lst[k][:, :, k:W], in_=lsb[:, :, k:W]).then_inc(
                    s_prl_init, 1
                )
                # right prep for d=k
                d = k
                if d > 0:
                    v.memset(rst[k][:, :, 0:d], 0.0)
                v.tensor_copy(
                    out=rst[k][:, :, d:W], in_=rsb[:, :, 0:W - d]
                ).then_inc(s_prr, 1)
            for d in range(K, D):
                k = d % K
                v.wait_ge(s_rdone[k], 16 * (d // K))
                v.memset(rst[k][:, :, d - K:d], 0.0)
                v.tensor_copy(
                    out=rst[k][:, :, d:W], in_=rsb[:, :, 0:W - d]
                ).then_inc(s_prr, 1)

        @block.gpsimd
        def _(g):
            # Rolling left masks: stage d%K advances from level d-K to d.
            for d in range(K, D):
                k = d % K
                g.wait_ge(s_ldone[k], 16 * (d // K))
                g.memset(lst[k][:, :, d - K:d], 0.0).then_inc(s_prl_roll, 1)

        @block.sync
        def _(s):
            s.dma_start(out=lsb[:], in_=left_t[:]).then_inc(s_lin, 16)
            for d in range(D):
                k = d % K
                if d < K:
                    s.wait_ge(s_prl_init, d + 1)
                else:
                    s.wait_ge(s_prl_roll, d - K + 1)
                s.dma_start(
                    out=out_t[0:C, d:d + 1, :, :], in_=lst[k][:]
                ).then_inc(s_ldone[k], 16)
            for k in range(K):
                s.wait_ge(s_ldone[k], 16 * uses[k])

        @block.scalar
        def _(a):
            a.dma_start(out=rsb[:], in_=right_t[:]).then_inc(s_rin, 16)
            for d in range(D):
                k = d % K
                a.wait_ge(s_prr, d + 1)
                a.dma_start(
                    out=out_t[C:2 * C, d:d + 1, :, :], in_=rst[k][:]
                ).then_inc(s_rdone[k], 16)
            for k in range(K):
                a.wait_ge(s_rdone[k], 16 * uses[k])

    return nc


_NC_CACHE: list = []


def _get_nc() -> bass.Bass:
    if not _NC_CACHE:
        _NC_CACHE.append(_build_nc())
    return _NC_CACHE[0]


def _shard(left: np.ndarray, right: np.ndarray) -> list:
    in_maps = []
    for b in range(B):
        for hh in range(H // HH):
            lc = np.ascontiguousarray(
                left[b, :, hh * HH:(hh + 1) * HH, :], dtype=np.float32
            ).reshape(ROWS, W)
            rc = np.ascontiguousarray(
                right[b, :, hh * HH:(hh + 1) * HH, :], dtype=np.float32
            ).reshape(ROWS, W)
            in_maps.append({"left": lc, "right": rc})
    return in_maps


def _run(left: np.ndarray, right: np.ndarray, **spmd_kwargs):
    nc = _get_nc()
    in_maps = _shard(left, right)
    res = run_bass_kernel_spmd(nc, in_maps, list(range(N_CORES)), **spmd_kwargs)
    out = np.empty((B, 2 * C, D, H, W), dtype=np.float32)
    core = 0
    for b in range(B):
        for hh in range(H // HH):
            out[b, :, :, hh * HH:(hh + 1) * HH, :] = res.results[core]["out"].reshape(
                2 * C, D, HH, W
            )
            core += 1
    return out, res


def kernel(left: np.ndarray, right: np.ndarray) -> np.ndarray:
    out, _ = _run(np.asarray(left), np.asarray(right))
    return out


# revision 6
# speedup vs baseline: 1.2051x; 1.0015x over previous
"""Stereo cost-volume construction kernel for Trainium2 (8 NeuronCores).

Problem: left, right: [B=4, C=32, H=64, W=128] f32 ->
         cost:        [B, 2C=64, D=48, H, W] f32
  cost[b, c,    d, h, w] = left [b, c, h, w]     if w >= d else 0
  cost[b, C+c,  d, h, w] = right[b, c, h, w - d] if w >= d else 0

Sharding: data-parallel over (b, h-half): core = b*2 + hh, each core owns
the full disparity range on a [C, 32, W] slice -> pure SPMD, no
communication, identical program on all 8 cores.

Per-core device strategy (memory-regime; all output bytes written
exactly once, full-width 512 KiB DMAs with 4 KiB descriptor runs):
  * K rotating SBUF stage buffers per half. A stage holds the complete
    output image for one disparity (zero prefix + data), so the output
    DMA is a plain full-width copy at peak descriptor efficiency.
  * left half: stage data never moves between uses (only the zero
    column-prefix grows), so reuse costs just a K-column memset
    (gpsimd); DVE seeds the K stages once from SBUF.
  * right half: data shifts with d, so DVE rebuilds each stage
    (memset of the K new prefix columns + shifted row copy).
  * left DMAs on the SP HWDGE queue, right DMAs on the Activation
    HWDGE queue; the two streams share the ~400 GB/s DMA engine pool.
"""

import numpy as np

import concourse.bass as bass
import concourse.mybir as mybir
from concourse.bass_utils import run_bass_kernel_spmd

B, C, H, W = 4, 32, 64, 128
D = 48
HH = H // 2          # rows of H per core
N_CORES = 8
ROWS = C * HH        # 1024 (c, h) rows per core
P = 128              # SBUF partitions
J = ROWS // P        # 8 rows per partition
K = 8                # stage buffers per half
F32 = mybir.dt.float32


def _build_nc() -> bass.Bass:
    nc = bass.Bass()

    left_t = nc.declare_dram_parameter("left", [ROWS, W], F32, isOutput=False)
    right_t = nc.declare_dram_parameter("right", [ROWS, W], F32, isOutput=False)
    out_t = nc.declare_dram_parameter("out", [2 * C, D, HH, W], F32, isOutput=True)

    lsb = nc.alloc_sbuf_tensor("lsb", [P, J, W], F32)
    rsb = nc.alloc_sbuf_tensor("rsb", [P, J, W], F32)
    lst = [nc.alloc_sbuf_tensor(f"lst{k}", [P, J, W], F32) for k in range(K)]
    rst = [nc.alloc_sbuf_tensor(f"rst{k}", [P, J, W], F32) for k in range(K)]

    s_lin = nc.alloc_semaphore("s_lin")
    s_rin = nc.alloc_semaphore("s_rin")
    s_prl_init = nc.alloc_semaphore("s_prl_init")  # DVE left seeds, d < K
    s_prl_roll = nc.alloc_semaphore("s_prl_roll")  # gpsimd left memsets, d >= K
    s_prr = nc.alloc_semaphore("s_prr")            # DVE right preps
    s_ldone = [nc.alloc_semaphore(f"s_ldone{k}") for k in range(K)]
    s_rdone = [nc.alloc_semaphore(f"s_rdone{k}") for k in range(K)]

    uses = [len(range(k, D, K)) for k in range(K)]

    with nc.Block() as block:

        @block.vector
        def _(v):
            # Seed left stages (data is d-invariant) and build right stages.
            # Interleave so both DMA queues start streaming ASAP.
            v.wait_ge(s_lin, 16)
            v.wait_ge(s_rin, 16)
            for k in range(K):
                # right prep for d=k first: the right stream is copy-gated
                d = k
                if d > 0:
                    v.memset(rst[k][:, :, 0:d], 0.0)
                v.tensor_copy(
                    out=rst[k][:, :, d:W], in_=rsb[:, :, 0:W - d]
                ).then_inc(s_prr, 1)
                if k > 0:
                    v.memset(lst[k][:, :, 0:k], 0.0)
                v.tensor_copy(out=lst[k][:, :, k:W], in_=lsb[:, :, k:W]).then_inc(
                    s_prl_init, 1
                )
            for d in range(K, D):
                k = d % K
                v.wait_ge(s_rdone[k], 16 * (d // K))
                v.memset(rst[k][:, :, d - K:d], 0.0)
                v.tensor_copy(
                    out=rst[k][:, :, d:W], in_=rsb[:, :, 0:W - d]
                ).then_inc(s_prr, 1)

        @block.gpsimd
        def _(g):
            # Rolling left masks: stage d%K advances from level d-K to d.
            for d in range(K, D):
                k = d % K
                g.wait_ge(s_ldone[k], 16 * (d // K))
                g.memset(lst[k][:, :, d - K:d], 0.0).then_inc(s_prl_roll, 1)

        @block.sync
        def _(s):
            s.dma_start(out=lsb[:], in_=left_t[:]).then_inc(s_lin, 16)
            for d in range(D):
                k = d % K
                if d < K:
                    s.wait_ge(s_prl_init, d + 1)
                else:
                    s.wait_ge(s_prl_roll, d - K + 1)
                s.dma_start(
                    out=out_t[0:C, d:d + 1, :, :], in_=lst[k][:]
                ).then_inc(s_ldone[k], 16)
            for k in range(K):
                s.wait_ge(s_ldone[k], 16 * uses[k])

        @block.scalar
        def _(a):
            a.dma_start(out=rsb[:], in_=right_t[:]).then_inc(s_rin, 16)
            for d in range(D):
                k = d % K
                a.wait_ge(s_prr, d + 1)
                a.dma_start(
                    out=out_t[C:2 * C, d:d + 1, :, :], in_=rst[k][:]
                ).then_inc(s_rdone[k], 16)
            for k in range(K):
                a.wait_ge(s_rdone[k], 16 * uses[k])

    return nc


_NC_CACHE: list = []


def _get_nc() -> bass.Bass:
    if not _NC_CACHE:
        _NC_CACHE.append(_build_nc())
    return _NC_CACHE[0]


def _shard(left: np.ndarray, right: np.ndarray) -> list:
    in_maps = []
    for b in range(B):
        for hh in range(H // HH):
            lc = np.ascontiguousarray(
                left[b, :, hh * HH:(hh + 1) * HH, :], dtype=np.float32
            ).reshape(ROWS, W)
            rc = np.ascontiguousarray(
                right[b, :, hh * HH:(hh + 1) * HH, :], dtype=np.float32
            ).reshape(ROWS, W)
            in_maps.append({"left": lc, "right": rc})
    return in_maps


def _run(left: np.ndarray, right: np.ndarray, **spmd_kwargs):
    nc = _get_nc()
    in_maps = _shard(left, right)
    res = run_bass_kernel_spmd(nc, in_maps, list(range(N_CORES)), **spmd_kwargs)
    out = np.empty((B, 2 * C, D, H, W), dtype=np.float32)
    core = 0
    for b in range(B):
        for hh in range(H // HH):
            out[b, :, :, hh * HH:(hh + 1) * HH, :] = res.results[core]["out"].reshape(
                2 * C, D, HH, W
            )
            core += 1
    return out, res


def kernel(left: np.ndarray, right: np.ndarray) -> np.ndarray:
    out, _ = _run(np.asarray(left), np.asarray(right))
    return out
